# revision 55
# baseline (speedup 1.0000x reference)
"""Single-head causal attention on 8 trn2 NeuronCores (Bass/Tile).

Problem: x [4, 4096, 768] f32; Wk/Wq/Wv [768, 64]; out = softmax(causal(q k^T/8)) v.

Sharding: 8 cores = 4 batches x 2 cores. Per batch the 32 query tiles
(128 rows each) are split between its 2 cores so that BOTH cores run the
IDENTICAL program: slot s (s=0..15) processes one q-tile against a key
prefix of exactly L[s] = 256*(s+1) local keys.  Core h=0 takes global
q-tiles {0,3,4,7,8,...}; core h=1 takes {1,2,5,6,9,...} and gets its x
PERMUTED on host (128-row tile pairs 2a<->2a+1 swapped) so the slot->tile
mapping is position-identical across cores.  Since every prefix length is
a multiple of 256 and the permutation only swaps inside 256-row pairs,
the KEY SET of every slot is exactly the causally-needed set; only the
last 256 keys of each slot need a mask, which the host precomputes.

Per-core kernel: project kT/vT/qT with PE (x arrives pre-transposed),
scoresT = K q^T per 128-key chunk (keys on partitions), exp on ScalarE
(no max subtraction: scores ~ N(0,1), fp32 exp is exact-safe), then
out^T accumulated as V_aug^T @ W with a ones-column in V_aug providing
softmax denominators for free.  All matmul operands are float32r
(1 cycle/col at N>=256 vs 4 for plain fp32).
"""

import functools
import os
import sys

import ml_dtypes
import numpy as np

if "/opt/trn_rl_repo" not in sys.path:
    sys.path.insert(0, "/opt/trn_rl_repo")

B, T, C, H = 4, 4096, 768, 64
# x + projection weights in bf16 halves the dominant DMA stream (memory-
# bound problem); projections still accumulate in fp32 on the PE.
X_BF16 = os.environ.get("X_BF16", "1") == "1"
NCHUNK = C // 128          # 6 embedding chunks
NSLOT = 16                 # q-tiles per core
NPAIR = 8                  # slot pairs / xT pieces
NEG = -30000.0

# local q-tile index per slot: even s -> 2s, odd s -> 2s+1
LTS = [2 * s + (s % 2) for s in range(NSLOT)]


def _build_program():
    import concourse.bass as bass
    import concourse.tile as tile
    from concourse import mybir
    from contextlib import ExitStack

    f32r = mybir.dt.float32r
    f32 = mybir.dt.float32
    EXP = mybir.ActivationFunctionType.Exp

    nc = bass.Bass(trn_type="TRN2", target_bir_lowering=False, debug=False)

    # host-prepared layouts (p = partition index)
    bf16 = mybir.dt.bfloat16
    xdt = bf16 if X_BF16 else f32r
    xT = nc.dram_tensor("xT", [128, NCHUNK, T], xdt, kind="ExternalInput").ap()
    wkv = nc.dram_tensor("wkv", [128, NCHUNK, 128], xdt, kind="ExternalInput").ap()
    wq = nc.dram_tensor("wq", [128, NCHUNK, H], xdt, kind="ExternalInput").ap()
    maskT = nc.dram_tensor(
        "maskT", [128, NSLOT, 2, 128], bf16, kind="ExternalInput"
    ).ap()
    ident_d = nc.dram_tensor("ident", [64, 64], f32r, kind="ExternalInput").ap()
    # row 0 = zeros, row 1 = ones
    consts_d = nc.dram_tensor("consts", [2, 256], f32r, kind="ExternalInput").ap()
    # output stays transposed [h, slot*128+q] with the softmax-denominator
    # row appended (row 64); host divides + untransposes for free
    out_d = nc.dram_tensor(
        "out", [H + 1, NSLOT * 128], f32r, kind="ExternalOutput"
    ).ap()

    with ExitStack() as ctx:
        tc = ctx.enter_context(tile.TileContext(nc))
        const = ctx.enter_context(tc.tile_pool(name="const", bufs=1))
        xp_pool = ctx.enter_context(tc.tile_pool(name="xp", bufs=4))
        wt_pool = ctx.enter_context(tc.tile_pool(name="wt", bufs=4))
        sb_misc = ctx.enter_context(tc.tile_pool(name="misc", bufs=4))
        ps_kv = ctx.enter_context(tc.tile_pool(name="pskv", bufs=1, space="PSUM"))
        ps_sc = ctx.enter_context(tc.tile_pool(name="pssc", bufs=2, space="PSUM"))
        ps_out = ctx.enter_context(tc.tile_pool(name="psout", bufs=3, space="PSUM"))

        wkv_s = const.tile([128, NCHUNK, 128], xdt)
        nc.sync.dma_start(out=wkv_s, in_=wkv)
        wq_s = const.tile([128, NCHUNK, H], xdt)
        nc.sync.dma_start(out=wq_s, in_=wq)
        # mask slices are DMA'd per-piece (inside the loop) so the 2MB
        # transfer doesn't delay the first x pieces at startup
        mask_s = const.tile([128, NSLOT, 2, 128], bf16)
        ident = const.tile([64, 64], f32r)
        nc.sync.dma_start(out=ident, in_=ident_d)

        kTs = const.tile([64, T], f32r)            # keys^T, local key order
        qTs = const.tile([64, NSLOT * 128], f32r)  # queries^T, slot order
        vaug = const.tile([128, T // 128, H + 1], f32r)  # V rows + ones col
        ones_row = consts_d[1:2, :]
        nc.sync.dma_start(
            out=vaug[:, :, H : H + 1],
            in_=bass.AP(tensor=ones_row.tensor, offset=ones_row.offset,
                        ap=[[0, 128], [1, T // 128], [0, 1]]),
        )
        zeros_s = const.tile([128, 2, 128], f32r)
        zrow = consts_d[0:1, :]
        nc.sync.dma_start(
            out=zeros_s,
            in_=bass.AP(tensor=zrow.tensor, offset=zrow.offset,
                        ap=[[0, 128], [128, 2], [1, 128]]),
        )



        for p in range(NPAIR):
            # ---- load xT piece p: [128, 6, 512] (keys 512p..512p+512) ----
            xp = xp_pool.tile([128, NCHUNK, 512], xdt, tag="xp")
            nc.sync.dma_start(out=xp, in_=xT[:, :, p * 512 : (p + 1) * 512])
            nc.sync.dma_start(
                out=mask_s[:, 2 * p : 2 * p + 2, :, :],
                in_=maskT[:, 2 * p : 2 * p + 2, :, :],
            )

            # ---- kv projection: [Wk|Wv]^T @ x^T -> [128, 512] ----
            kv_ps = ps_kv.tile([128, 512], f32, tag="kv")
            for c in range(NCHUNK):
                nc.tensor.matmul(
                    kv_ps,
                    lhsT=wkv_s[:, c, :],
                    rhs=xp[:, c, :],
                    start=(c == 0),
                    stop=(c == NCHUNK - 1),
                )
            nc.vector.tensor_copy(kTs[:, p * 512 : (p + 1) * 512], kv_ps[0:64, :])
            vts = sb_misc.tile([64, 512], f32r, tag="vts")
            nc.vector.tensor_copy(vts, kv_ps[64:128, :])
            vt_ps = ps_out.tile([128, 4, H], f32r, tag="oT")
            for j in range(4):
                nc.tensor.transpose(vt_ps[:, j, :], vts[:, j * 128 : (j + 1) * 128],
                                    ident[0:64, 0:64])
            nc.vector.tensor_copy(vaug[:, 4 * p : 4 * p + 4, 0:H], vt_ps)

            # ---- q projection for my 2 tiles in this piece (local tiles
            # 4p and 4p+3 -> slots 2p, 2p+1), N=256 via 2-range AP ----
            qq_ps = ps_out.tile([64, 256], f32, tag="oT")
            for c in range(NCHUNK):
                base = xp[:, c, 0:128]
                q_rhs = bass.AP(
                    tensor=base.tensor,
                    offset=base.offset,
                    ap=[base.ap[0], [384, 2], [1, 128]],
                )
                nc.tensor.matmul(
                    qq_ps,
                    lhsT=wq_s[:, c, :],
                    rhs=q_rhs,
                    start=(c == 0),
                    stop=(c == NCHUNK - 1),
                )
            nc.vector.tensor_copy(qTs[:, p * 256 : (p + 1) * 256], qq_ps)

            # ---- attention for slot pair (2p, 2p+1) ----
            # scoresT computed in groups of 4 key-chunks sharing one 2-bank
            # PSUM tile so exp is a single big ACT instruction per group.
            khi = 4 * (p + 1)   # chunks for slot 2p+1
            klo = khi - 2       # chunks for slot 2p
            s0, s1 = 2 * p, 2 * p + 1
            outT_ps = ps_out.tile([H + 1, 256], f32, tag="oT")

            def emit_scores_exp(g):
                scg = ps_sc.tile([128, 4, 256], f32, tag="sc")
                for j in range(4):
                    kc = 4 * g + j
                    nc.tensor.matmul(
                        scg[:, j, :],
                        lhsT=kTs[:, kc * 128 : (kc + 1) * 128],
                        rhs=qTs[:, p * 256 : (p + 1) * 256],
                        start=True,
                        stop=True,
                        skip_group_check=True,
                    )
                if g == p:
                    # window masks live in the last group: slot 2p on the
                    # left halves of quarters 0-1, slot 2p+1 on the right
                    # halves of quarters 2-3
                    nc.vector.tensor_add(
                        scg[:, 0:2, 0:128], scg[:, 0:2, 0:128],
                        mask_s[:, s0, :, :],
                    )
                    nc.vector.tensor_add(
                        scg[:, 2:4, 128:256], scg[:, 2:4, 128:256],
                        mask_s[:, s1, :, :],
                    )
                wt = wt_pool.tile([128, 4, 256], f32r, tag="wt")
                nc.scalar.activation(wt, scg, EXP)
                if g == p:
                    # slot 2p's context ended at klo: zero its (garbage)
                    # halves in quarters 2-3 (ACT keeps deps single-engine)
                    nc.scalar.copy(wt[:, 2:4, 0:128], zeros_s)
                return wt

            def emit_av(g, wt):
                for j in range(4):
                    kc = 4 * g + j
                    nc.tensor.matmul(
                        outT_ps,
                        lhsT=vaug[:, kc, :],
                        rhs=wt[:, j, :],
                        start=(kc == 0),
                        stop=(kc == khi - 1),
                        skip_group_check=True,
                    )

            # one-group software pipeline: scores(g+1) is emitted before
            # AV(g) so PE streams scores while ACT exps the previous group
            wt_prev = emit_scores_exp(0)
            for g in range(1, p + 1):
                wt_g = emit_scores_exp(g)
                emit_av(g - 1, wt_prev)
                wt_prev = wt_g
            emit_av(p, wt_prev)

            # ---- store (normalization happens on host) ----
            outT_s = sb_misc.tile([H + 1, 256], f32r, tag="oTs")
            nc.vector.tensor_copy(outT_s, outT_ps)
            nc.sync.dma_start(out=out_d[:, p * 256 : (p + 1) * 256], in_=outT_s)

    _split_matmul_waits(nc, mybir)
    return nc


def _split_matmul_waits(nc, mybir):
    """Several TRN2 instruction structs carry only ONE sync-wait slot
    (walrus: "Too many sync wait commands").  Hoist extra waits onto a
    chain of InstNoOps inserted immediately before, on the same engine —
    in-order execution preserves the semantics."""
    k = 0
    skip = {"InstAllEngineBarrier", "InstNoOp"}
    for f in nc.m.functions:
        for blk in f.blocks:
            il = blk.instructions
            i = 0
            while i < len(il):
                inst = il[i]
                if type(inst).__name__ not in skip:
                    si = inst.sync_info
                    waits = list(si.on_wait) if si is not None and si.on_wait else []
                    if len(waits) > 1:
                        for w in waits[:-1]:
                            nop = mybir.InstNoOp(
                                name=f"I-waitfix-{k}",
                                engine=inst.engine,
                                sync_info=mybir.SyncInfo(
                                    on_wait=[w], on_update=[]
                                ),
                            )
                            k += 1
                            il.insert(i, nop)
                            i += 1
                        inst.sync_info = mybir.SyncInfo(
                            on_wait=waits[-1:], on_update=list(si.on_update or [])
                        )
                i += 1


@functools.lru_cache(maxsize=1)
def _get_program():
    return _build_program()


def _diag_block():
    j = np.arange(128)[:, None]
    i = np.arange(128)[None, :]
    return np.where(j <= i, 0.0, NEG).astype(np.float32)


def _host_inputs(x, Wk, Wq, Wv):
    """Build per-core input dicts."""
    diag = _diag_block()
    full = np.zeros((128, 128), np.float32)
    masked = np.full((128, 128), NEG, np.float32)

    xdt = ml_dtypes.bfloat16 if X_BF16 else np.float32
    wkv_h = (
        np.concatenate([Wk, Wv], axis=1)
        .reshape(NCHUNK, 128, 128)
        .transpose(1, 0, 2)
        .astype(xdt)
    )
    wq_h = (
        (Wq / np.sqrt(H)).reshape(NCHUNK, 128, H).transpose(1, 0, 2).astype(xdt)
    )

    in_maps = []
    for core in range(8):
        b, h = core // 2, core % 2
        xt = np.asarray(x[b]).reshape(T // 128, 128, C)
        if h == 1:
            perm = np.arange(T // 128) ^ 1
            xt = xt[perm]
        # [p, c, t] = xperm[t, c*128+p]
        xT_h = (
            xt.reshape(T, C).T.reshape(NCHUNK, 128, T).transpose(1, 0, 2)
            .astype(xdt)
        )
        mask_h = np.empty((128, NSLOT, 2, 128), ml_dtypes.bfloat16)
        for s in range(NSLOT):
            qg = LTS[s] ^ h
            for kk in range(2):
                kg = (2 * s + kk) ^ h
                if kg == qg:
                    blk = diag
                elif kg < qg:
                    blk = full
                else:
                    blk = masked
                mask_h[:, s, kk, :] = blk
        in_maps.append(
            {"xT": xT_h, "wkv": wkv_h, "wq": wq_h, "maskT": mask_h,
             "ident": np.eye(64, dtype=np.float32),
             "consts": np.stack([np.zeros(256, np.float32),
                                 np.ones(256, np.float32)])}
        )
    return in_maps


def _unshard(results):
    out = np.empty((B, T, H), np.float32)
    for core in range(8):
        b, h = core // 2, core % 2
        oc = results[core]["out"]            # [H+1, NSLOT*128] unnormalized
        oc = (oc[:H] / oc[H : H + 1]).reshape(H, NSLOT, 128)
        ob = out[b].reshape(T // 128, 128, H)
        for s in range(NSLOT):
            ob[LTS[s] ^ h] = oc[:, s, :].T
    return out


def kernel(x, Wk, Wq, Wv):
    from concourse import bass_utils

    nc = _get_program()
    in_maps = _host_inputs(
        np.asarray(x, np.float32),
        np.asarray(Wk, np.float32),
        np.asarray(Wq, np.float32),
        np.asarray(Wv, np.float32),
    )
    res = bass_utils.run_bass_kernel_spmd(nc, in_maps, core_ids=list(range(8)))
    return _unshard(res.results)


# revision 58
# speedup vs baseline: 1.0154x; 1.0154x over previous
"""Single-head causal attention on 8 trn2 NeuronCores (Bass/Tile).

Problem: x [4, 4096, 768] f32; Wk/Wq/Wv [768, 64]; out = softmax(causal(q k^T/8)) v.

Sharding: 8 cores = 4 batches x 2 cores. Per batch the 32 query tiles
(128 rows each) are split between its 2 cores so that BOTH cores run the
IDENTICAL program: slot s (s=0..15) processes one q-tile against a key
prefix of exactly L[s] = 256*(s+1) local keys.  Core h=0 takes global
q-tiles {0,3,4,7,8,...}; core h=1 takes {1,2,5,6,9,...} and gets its x
PERMUTED on host (128-row tile pairs 2a<->2a+1 swapped) so the slot->tile
mapping is position-identical across cores.  Since every prefix length is
a multiple of 256 and the permutation only swaps inside 256-row pairs,
the KEY SET of every slot is exactly the causally-needed set; only the
last 256 keys of each slot need a mask, which the host precomputes.

Per-core kernel: project kT/vT/qT with PE (x arrives pre-transposed),
scoresT = K q^T per 128-key chunk (keys on partitions), exp on ScalarE
(no max subtraction: scores ~ N(0,1), fp32 exp is exact-safe), then
out^T accumulated as V_aug^T @ W with a ones-column in V_aug providing
softmax denominators for free.  All matmul operands are float32r
(1 cycle/col at N>=256 vs 4 for plain fp32).
"""

import functools
import os
import sys

import ml_dtypes
import numpy as np

if "/opt/trn_rl_repo" not in sys.path:
    sys.path.insert(0, "/opt/trn_rl_repo")

B, T, C, H = 4, 4096, 768, 64
# x + projection weights in bf16 halves the dominant DMA stream (memory-
# bound problem); projections still accumulate in fp32 on the PE.
X_BF16 = os.environ.get("X_BF16", "1") == "1"
NCHUNK = C // 128          # 6 embedding chunks
NSLOT = 16                 # q-tiles per core
NPAIR = 8                  # slot pairs / xT pieces
NEG = -30000.0

# local q-tile index per slot: even s -> 2s, odd s -> 2s+1
LTS = [2 * s + (s % 2) for s in range(NSLOT)]


def _build_program():
    import concourse.bass as bass
    import concourse.tile as tile
    from concourse import mybir
    from contextlib import ExitStack

    f32r = mybir.dt.float32r
    f32 = mybir.dt.float32
    EXP = mybir.ActivationFunctionType.Exp

    nc = bass.Bass(trn_type="TRN2", target_bir_lowering=False, debug=False)

    # host-prepared layouts (p = partition index)
    bf16 = mybir.dt.bfloat16
    xdt = bf16 if X_BF16 else f32r
    xT = nc.dram_tensor("xT", [128, NCHUNK, T], xdt, kind="ExternalInput").ap()
    wkv = nc.dram_tensor("wkv", [128, NCHUNK, 128], xdt, kind="ExternalInput").ap()
    wq = nc.dram_tensor("wq", [128, NCHUNK, H], xdt, kind="ExternalInput").ap()
    maskT = nc.dram_tensor(
        "maskT", [128, NSLOT, 2, 128], bf16, kind="ExternalInput"
    ).ap()
    ident_d = nc.dram_tensor("ident", [64, 64], f32r, kind="ExternalInput").ap()
    # row 0 = zeros, row 1 = ones
    consts_d = nc.dram_tensor("consts", [2, 256], f32r, kind="ExternalInput").ap()
    # output stays transposed [h, slot*128+q] with the softmax-denominator
    # row appended (row 64); host divides + untransposes for free
    out_d = nc.dram_tensor(
        "out", [H + 1, NSLOT * 128], f32r, kind="ExternalOutput"
    ).ap()

    with ExitStack() as ctx:
        tc = ctx.enter_context(tile.TileContext(nc))
        const = ctx.enter_context(tc.tile_pool(name="const", bufs=1))
        xp_pool = ctx.enter_context(tc.tile_pool(name="xp", bufs=4))
        wt_pool = ctx.enter_context(tc.tile_pool(name="wt", bufs=4))
        sb_misc = ctx.enter_context(tc.tile_pool(name="misc", bufs=4))
        ps_kv = ctx.enter_context(tc.tile_pool(name="pskv", bufs=1, space="PSUM"))
        ps_sc = ctx.enter_context(tc.tile_pool(name="pssc", bufs=2, space="PSUM"))
        ps_out = ctx.enter_context(tc.tile_pool(name="psout", bufs=3, space="PSUM"))

        wkv_s = const.tile([128, NCHUNK, 128], xdt)
        nc.sync.dma_start(out=wkv_s, in_=wkv)
        wq_s = const.tile([128, NCHUNK, H], xdt)
        nc.sync.dma_start(out=wq_s, in_=wq)
        # mask slices are DMA'd per-piece (inside the loop) so the 2MB
        # transfer doesn't delay the first x pieces at startup
        mask_s = const.tile([128, NSLOT, 2, 128], bf16)
        ident = const.tile([64, 64], f32r)
        nc.sync.dma_start(out=ident, in_=ident_d)

        kTs = const.tile([64, T], f32r)            # keys^T, local key order
        qTs = const.tile([64, NSLOT * 128], f32r)  # queries^T, slot order
        vaug = const.tile([128, T // 128, H + 1], f32r)  # V rows + ones col
        ones_row = consts_d[1:2, :]
        nc.sync.dma_start(
            out=vaug[:, :, H : H + 1],
            in_=bass.AP(tensor=ones_row.tensor, offset=ones_row.offset,
                        ap=[[0, 128], [1, T // 128], [0, 1]]),
        )
        zeros_s = const.tile([128, 2, 128], f32r)
        zrow = consts_d[0:1, :]
        nc.sync.dma_start(
            out=zeros_s,
            in_=bass.AP(tensor=zrow.tensor, offset=zrow.offset,
                        ap=[[0, 128], [128, 2], [1, 128]]),
        )
        # preload the exp table set during the startup DMA window
        warm = sb_misc.tile([1, 2], f32r, tag="warm")
        nc.scalar.activation(warm, zeros_s[0:1, 0, 0:2], EXP)



        for p in range(NPAIR):
            # ---- load xT piece p: [128, 6, 512] (keys 512p..512p+512) ----
            xp = xp_pool.tile([128, NCHUNK, 512], xdt, tag="xp")
            nc.sync.dma_start(out=xp, in_=xT[:, :, p * 512 : (p + 1) * 512])
            nc.sync.dma_start(
                out=mask_s[:, 2 * p : 2 * p + 2, :, :],
                in_=maskT[:, 2 * p : 2 * p + 2, :, :],
            )

            # ---- kv projection: [Wk|Wv]^T @ x^T -> [128, 512] ----
            kv_ps = ps_kv.tile([128, 512], f32, tag="kv")
            for c in range(NCHUNK):
                nc.tensor.matmul(
                    kv_ps,
                    lhsT=wkv_s[:, c, :],
                    rhs=xp[:, c, :],
                    start=(c == 0),
                    stop=(c == NCHUNK - 1),
                )
            nc.vector.tensor_copy(kTs[:, p * 512 : (p + 1) * 512], kv_ps[0:64, :])
            vts = sb_misc.tile([64, 512], f32r, tag="vts")
            nc.vector.tensor_copy(vts, kv_ps[64:128, :])
            vt_ps = ps_out.tile([128, 4, H], f32r, tag="oT")
            for j in range(4):
                nc.tensor.transpose(vt_ps[:, j, :], vts[:, j * 128 : (j + 1) * 128],
                                    ident[0:64, 0:64])
            nc.vector.tensor_copy(vaug[:, 4 * p : 4 * p + 4, 0:H], vt_ps)

            # ---- q projection for my 2 tiles in this piece (local tiles
            # 4p and 4p+3 -> slots 2p, 2p+1), N=256 via 2-range AP ----
            qq_ps = ps_out.tile([64, 256], f32, tag="oT")
            for c in range(NCHUNK):
                base = xp[:, c, 0:128]
                q_rhs = bass.AP(
                    tensor=base.tensor,
                    offset=base.offset,
                    ap=[base.ap[0], [384, 2], [1, 128]],
                )
                nc.tensor.matmul(
                    qq_ps,
                    lhsT=wq_s[:, c, :],
                    rhs=q_rhs,
                    start=(c == 0),
                    stop=(c == NCHUNK - 1),
                )
            nc.vector.tensor_copy(qTs[:, p * 256 : (p + 1) * 256], qq_ps)

            # ---- attention for slot pair (2p, 2p+1) ----
            # scoresT computed in groups of 4 key-chunks sharing one 2-bank
            # PSUM tile so exp is a single big ACT instruction per group.
            khi = 4 * (p + 1)   # chunks for slot 2p+1
            klo = khi - 2       # chunks for slot 2p
            s0, s1 = 2 * p, 2 * p + 1
            outT_ps = ps_out.tile([H + 1, 256], f32, tag="oT")

            def emit_scores_exp(g):
                scg = ps_sc.tile([128, 4, 256], f32, tag="sc")
                for j in range(4):
                    kc = 4 * g + j
                    nc.tensor.matmul(
                        scg[:, j, :],
                        lhsT=kTs[:, kc * 128 : (kc + 1) * 128],
                        rhs=qTs[:, p * 256 : (p + 1) * 256],
                        start=True,
                        stop=True,
                        skip_group_check=True,
                    )
                if g == p:
                    # window masks live in the last group: slot 2p on the
                    # left halves of quarters 0-1, slot 2p+1 on the right
                    # halves of quarters 2-3
                    nc.vector.tensor_add(
                        scg[:, 0:2, 0:128], scg[:, 0:2, 0:128],
                        mask_s[:, s0, :, :],
                    )
                    nc.vector.tensor_add(
                        scg[:, 2:4, 128:256], scg[:, 2:4, 128:256],
                        mask_s[:, s1, :, :],
                    )
                wt = wt_pool.tile([128, 4, 256], f32r, tag="wt")
                nc.scalar.activation(wt, scg, EXP)
                if g == p:
                    # slot 2p's context ended at klo: zero its (garbage)
                    # halves in quarters 2-3 (DVE; the waitfix pass splits
                    # the consuming matmul's multi-engine waits)
                    nc.vector.tensor_copy(wt[:, 2:4, 0:128], zeros_s)
                return wt

            def emit_av(g, wt):
                for j in range(4):
                    kc = 4 * g + j
                    nc.tensor.matmul(
                        outT_ps,
                        lhsT=vaug[:, kc, :],
                        rhs=wt[:, j, :],
                        start=(kc == 0),
                        stop=(kc == khi - 1),
                        skip_group_check=True,
                    )

            # one-group software pipeline: scores(g+1) is emitted before
            # AV(g) so PE streams scores while ACT exps the previous group
            wt_prev = emit_scores_exp(0)
            for g in range(1, p + 1):
                wt_g = emit_scores_exp(g)
                emit_av(g - 1, wt_prev)
                wt_prev = wt_g
            emit_av(p, wt_prev)

            # ---- store (normalization happens on host) ----
            outT_s = sb_misc.tile([H + 1, 256], f32r, tag="oTs")
            nc.vector.tensor_copy(outT_s, outT_ps)
            nc.sync.dma_start(out=out_d[:, p * 256 : (p + 1) * 256], in_=outT_s)

    _split_matmul_waits(nc, mybir)
    return nc


def _split_matmul_waits(nc, mybir):
    """Several TRN2 instruction structs carry only ONE sync-wait slot
    (walrus: "Too many sync wait commands").  Hoist extra waits onto a
    chain of InstNoOps inserted immediately before, on the same engine —
    in-order execution preserves the semantics."""
    k = 0
    skip = {"InstAllEngineBarrier", "InstNoOp"}
    for f in nc.m.functions:
        for blk in f.blocks:
            il = blk.instructions
            i = 0
            while i < len(il):
                inst = il[i]
                if type(inst).__name__ not in skip:
                    si = inst.sync_info
                    waits = list(si.on_wait) if si is not None and si.on_wait else []
                    if len(waits) > 1:
                        for w in waits[:-1]:
                            nop = mybir.InstNoOp(
                                name=f"I-waitfix-{k}",
                                engine=inst.engine,
                                sync_info=mybir.SyncInfo(
                                    on_wait=[w], on_update=[]
                                ),
                            )
                            k += 1
                            il.insert(i, nop)
                            i += 1
                        inst.sync_info = mybir.SyncInfo(
                            on_wait=waits[-1:], on_update=list(si.on_update or [])
                        )
                i += 1


@functools.lru_cache(maxsize=1)
def _get_program():
    return _build_program()


def _diag_block():
    j = np.arange(128)[:, None]
    i = np.arange(128)[None, :]
    return np.where(j <= i, 0.0, NEG).astype(np.float32)


def _host_inputs(x, Wk, Wq, Wv):
    """Build per-core input dicts."""
    diag = _diag_block()
    full = np.zeros((128, 128), np.float32)
    masked = np.full((128, 128), NEG, np.float32)

    xdt = ml_dtypes.bfloat16 if X_BF16 else np.float32
    wkv_h = (
        np.concatenate([Wk, Wv], axis=1)
        .reshape(NCHUNK, 128, 128)
        .transpose(1, 0, 2)
        .astype(xdt)
    )
    wq_h = (
        (Wq / np.sqrt(H)).reshape(NCHUNK, 128, H).transpose(1, 0, 2).astype(xdt)
    )

    in_maps = []
    for core in range(8):
        b, h = core // 2, core % 2
        xt = np.asarray(x[b]).reshape(T // 128, 128, C)
        if h == 1:
            perm = np.arange(T // 128) ^ 1
            xt = xt[perm]
        # [p, c, t] = xperm[t, c*128+p]
        xT_h = (
            xt.reshape(T, C).T.reshape(NCHUNK, 128, T).transpose(1, 0, 2)
            .astype(xdt)
        )
        mask_h = np.empty((128, NSLOT, 2, 128), ml_dtypes.bfloat16)
        for s in range(NSLOT):
            qg = LTS[s] ^ h
            for kk in range(2):
                kg = (2 * s + kk) ^ h
                if kg == qg:
                    blk = diag
                elif kg < qg:
                    blk = full
                else:
                    blk = masked
                mask_h[:, s, kk, :] = blk
        in_maps.append(
            {"xT": xT_h, "wkv": wkv_h, "wq": wq_h, "maskT": mask_h,
             "ident": np.eye(64, dtype=np.float32),
             "consts": np.stack([np.zeros(256, np.float32),
                                 np.ones(256, np.float32)])}
        )
    return in_maps


def _unshard(results):
    out = np.empty((B, T, H), np.float32)
    for core in range(8):
        b, h = core // 2, core % 2
        oc = results[core]["out"]            # [H+1, NSLOT*128] unnormalized
        oc = (oc[:H] / oc[H : H + 1]).reshape(H, NSLOT, 128)
        ob = out[b].reshape(T // 128, 128, H)
        for s in range(NSLOT):
            ob[LTS[s] ^ h] = oc[:, s, :].T
    return out


def kernel(x, Wk, Wq, Wv):
    from concourse import bass_utils

    nc = _get_program()
    in_maps = _host_inputs(
        np.asarray(x, np.float32),
        np.asarray(Wk, np.float32),
        np.asarray(Wq, np.float32),
        np.asarray(Wv, np.float32),
    )
    res = bass_utils.run_bass_kernel_spmd(nc, in_maps, core_ids=list(range(8)))
    return _unshard(res.results)
